# revision 4
# baseline (speedup 1.0000x reference)
"""Trainium2 Bass kernel for nn_Attention_18176301596931.

Dense GQA attention layer (B=1, S=2048, D=2048, 32 Q heads / 8 KV heads,
HD=64, interleaved RoPE, causal softmax) tensor-parallel over 8 NeuronCores:
core i owns Q heads 4i..4i+3 and KV head i. Each core computes its heads'
QKV projection, RoPE, attention, and a partial output projection over its
256 columns of wo; the host sums the 8 partial outputs.

All matmuls run as float32r (full-rate on TRN2 when the moving free dim is
>= 256) with fp32 PSUM accumulation:
  - qkvT[e,s]   = wqkvT.T @ xT          (contraction over d, 16 chunks)
  - scoresT     = kT_chunk.T @ qT_win   ([sk=128, sq=512] blocks)
  - pT          = exp(0.125*scoresT)    (ACT, straight from PSUM; causal
                                         mask = 0/1 multiply, diag blocks)
  - yT_aug      = v_aug.T @ pT          (v augmented with a ones column so
                                         row 64 accumulates softmax sums)
  - out_part    = yT.T @ woT            (partial over this core's e range)

Softmax skips max-subtraction (logits ~ N(0,1), exp is safe in fp32; the
result is mathematically identical). RoPE pairs are de-interleaved by
permuting wqkv's q/k rows host-side, so on-device RoPE is three
128-partition DVE ops per tile (out = t*A + swap(t)*B).
"""

import numpy as np
import jax
from jax.sharding import Mesh, PartitionSpec
from jax.experimental.shard_map import shard_map

import concourse.bass as bass
import concourse.mybir as mybir
import concourse.tile as tile
from concourse.bacc import Bacc
from concourse import bass2jax
from concourse.bass2jax import (
    _bass_exec_p,
    install_neuronx_cc_hook,
    partition_id_tensor,
)

F32 = mybir.dt.float32
F32R = mybir.dt.float32r

B, S, D = 1, 2048, 2048
NH, NKV, HD = 32, 8, 64
N_CORES = 8
HQ = NH // N_CORES          # 4 q heads per core
EQ = HQ * HD                # 256 q columns per core
ECORE = EQ + 2 * HD         # 384 qkv columns per core
DO = D // 128               # 16 contraction chunks
SW = 512                    # seq window (matmul moving dim)
NG = S // SW                # 4 groups
NSB = S // 128              # 16 sk blocks
SCALE = 1.0 / np.sqrt(HD)


def _build(loop: int = 1) -> bass.Bass:
    nc = Bacc()
    x_ext = nc.declare_dram_parameter("xt", [128, DO, S], F32R, isOutput=False)
    w_ext = nc.declare_dram_parameter("wt", [128, DO, ECORE], F32R, isOutput=False)
    wo_ext = nc.declare_dram_parameter("wot", [128, 2, D], F32R, isOutput=False)
    a_ext = nc.declare_dram_parameter("ropea", [HD, S], F32R, isOutput=False)
    b_ext = nc.declare_dram_parameter("ropeb", [HD, S], F32R, isOutput=False)
    m_ext = nc.declare_dram_parameter("masks", [128, 4, SW], F32R, isOutput=False)
    id_ext = nc.declare_dram_parameter("ident", [HD, HD], F32R, isOutput=False)
    oc_ext = nc.declare_dram_parameter("onescol", [1, HD], F32R, isOutput=False)
    vo_ext = nc.declare_dram_parameter("vones", [128, NSB, 1], F32R, isOutput=False)
    out_ext = nc.declare_dram_parameter("out", [S, D], F32, isOutput=True)

    W1 = 256                  # phase-1 seq window (keeps xs buffers small)
    NW1 = S // W1

    with (
        nc.allow_low_precision(reason="float32r storage is intentional"),
        tile.TileContext(nc) as tc,
        tc.tile_pool(name="const", bufs=1) as constp,
        tc.tile_pool(name="persist", bufs=1) as persist,
        tc.tile_pool(name="xs", bufs=2) as xsp,
        tc.tile_pool(name="work", bufs=2) as work,
        tc.tile_pool(name="pt", bufs=3) as ptp,
        tc.tile_pool(name="outp", bufs=2) as outp,
        tc.tile_pool(name="ps", bufs=3, space="PSUM") as psp,
        tc.tile_pool(name="psy", bufs=2, space="PSUM") as psyp,
        tc.tile_pool(name="psb", bufs=1, space="PSUM") as psbp,
        tc.tile_pool(name="psv", bufs=1, space="PSUM") as psvp,
    ):
        ident = constp.tile([HD, HD], F32R)
        onescol = constp.tile([1, HD], F32R)
        ropea = constp.tile([HD, S], F32R)
        ropeb = constp.tile([HD, S], F32R)
        masks = constp.tile([128, 4, SW], F32R)
        wot = constp.tile([128, 2, D], F32R)
        wt_sb = constp.tile([128, DO, ECORE], F32R)
        nc.sync.dma_start(ident[:], id_ext[:])
        nc.sync.dma_start(onescol[:], oc_ext[:])
        nc.sync.dma_start(ropea[:], a_ext[:])
        nc.sync.dma_start(ropeb[:], b_ext[:])
        nc.sync.dma_start(masks[:], m_ext[:])
        nc.sync.dma_start(wot[:], wo_ext[:])
        nc.sync.dma_start(wt_sb[:], w_ext[:])

        def body(_i=None):
            # ---- Phase 1: QKV projection with RoPE fused into the PSUM
            # eviction. PSUM-source ops may shift base partitions, so the
            # real/imag swap halves are read straight out of PSUM. ----
            qh = [persist.tile([HD, S], F32R, name=f"qh{h}") for h in range(HQ)]
            kT = persist.tile([HD, S], F32R, name="kT")
            vT = persist.tile([HD, S], F32R, name="vT")

            def rope_evict(dst_cols, ps, r, w1cols):
                # dst = ps[r:r+64]*A + swap(ps[r:r+64])*B
                sw_t = work.tile([HD, W1], F32R, name="swtmp")
                nc.vector.tensor_mul(
                    out=sw_t[0:32, :], in0=ps[r + 32 : r + 64, :],
                    in1=ropeb[0:32, w1cols],
                )
                nc.vector.tensor_mul(
                    out=sw_t[32:64, :], in0=ps[r : r + 32, :],
                    in1=ropeb[32:64, w1cols],
                )
                nc.vector.tensor_mul(
                    out=dst_cols, in0=ps[r : r + HD, :], in1=ropea[:, w1cols]
                )
                nc.vector.tensor_add(out=dst_cols, in0=dst_cols, in1=sw_t[:])

            for w in range(NW1):
                cols = slice(w * W1, (w + 1) * W1)
                xs = xsp.tile([128, DO, W1], F32R, name="xs")
                nc.sync.dma_start(xs[:], x_ext[:, :, cols])
                for e in range(3):
                    ps = psp.tile([128, SW], F32, name="mm")[:, :W1]
                    for ko in range(DO):
                        nc.tensor.matmul(
                            ps,
                            wt_sb[:, ko, e * 128 : (e + 1) * 128],
                            xs[:, ko, :],
                            start=(ko == 0),
                            stop=(ko == DO - 1),
                        )
                    if e < 2:
                        for half in range(2):
                            h = 2 * e + half
                            rope_evict(qh[h][:, cols], ps, 64 * half, cols)
                    else:
                        rope_evict(kT[:, cols], ps, 0, cols)
                        nc.scalar.copy(vT[:, cols], ps[HD:128, :])

            # ---- Phase 2: transpose V into [sk, hd] layout + ones column ----
            v_sk = persist.tile([128, NSB, HD + 1], F32R, name="v_sk")
            nc.sync.dma_start(v_sk[:, :, HD : HD + 1], vo_ext[:])
            for c in range(NSB):
                pv = psvp.tile([128, HD], F32R, name="vt")
                nc.tensor.transpose(
                    pv[:], vT[:, c * 128 : (c + 1) * 128], ident[:]
                )
                nc.vector.tensor_copy(v_sk[:, c, 0:HD], pv[:])

            # ---- Phase 4: attention per (head, group) ----
            yT = [persist.tile([128, S], F32R, name=f"yT{c}") for c in range(2)]
            for h in range(HQ):
                for g in range(NG):
                    qw = qh[h][:, g * SW : (g + 1) * SW]
                    nblk = 4 * (g + 1)
                    psy = psyp.tile([HD + 1, SW], F32, name="y")
                    for b in range(nblk):
                        pss = psp.tile([128, SW], F32, name="mm")
                        nc.tensor.matmul(
                            pss[:],
                            kT[:, b * 128 : (b + 1) * 128],
                            qw,
                            start=True,
                            stop=True,
                        )
                        pt = ptp.tile([128, SW], F32R, name="pt")
                        nc.scalar.activation(
                            pt[:],
                            pss[:],
                            mybir.ActivationFunctionType.Exp,
                            scale=float(SCALE),
                        )
                        j = b - (nblk - 4)
                        if j >= 0:
                            nc.vector.tensor_mul(
                                out=pt[:], in0=pt[:], in1=masks[:, j, :]
                            )
                        nc.tensor.matmul(
                            psy[:],
                            v_sk[:, b, :],
                            pt[:],
                            start=(b == 0),
                            stop=(b == nblk - 1),
                        )
                    # normalize: yT = psy[0:64] * (1/sums) broadcast over hd
                    rec = work.tile([1, SW], F32R, name="rec")
                    nc.vector.reciprocal(rec[:], psy[HD : HD + 1, :])
                    psb = psbp.tile([HD, SW], F32, name="bc")
                    nc.tensor.matmul(psb[:], onescol[:], rec[:], start=True, stop=True)
                    bcs = work.tile([HD, SW], F32, name="bcs")
                    nc.scalar.copy(bcs[:], psb[:])
                    nc.vector.tensor_mul(
                        out=yT[h // 2][(h % 2) * HD : (h % 2) * HD + HD,
                                       g * SW : (g + 1) * SW],
                        in0=psy[0:HD, :],
                        in1=bcs[:],
                    )

            # ---- Phase 5: output projection (partial over this core's e) ----
            for sq in range(NSB):
                for do in range(NG):
                    pso = psp.tile([128, SW], F32, name="mm")
                    for c in range(2):
                        nc.tensor.matmul(
                            pso[:],
                            yT[c][:, sq * 128 : (sq + 1) * 128],
                            wot[:, c, do * SW : (do + 1) * SW],
                            start=(c == 0),
                            stop=(c == 1),
                        )
                    ot = outp.tile([128, SW], F32, name="ot")
                    if (sq + do) % 2 == 0:
                        nc.scalar.copy(ot[:], pso[:])
                    else:
                        nc.vector.tensor_copy(ot[:], pso[:])
                    nc.sync.dma_start(
                        out_ext[sq * 128 : (sq + 1) * 128, do * SW : (do + 1) * SW],
                        ot[:],
                    )

        if loop <= 1:
            body()
        else:
            with tc.For_i(0, loop, 1) as i:
                body(i)
    nc.finalize()
    return nc


class _CompiledSpmd:
    def __init__(self, nc: bass.Bass, n_cores: int = N_CORES):
        install_neuronx_cc_hook()
        self.nc = nc
        self.n_cores = n_cores
        partition_name = nc.partition_id_tensor.name if nc.partition_id_tensor else None

        in_names, out_names, out_avals, zero_shapes = [], [], [], []
        for alloc in nc.m.functions[0].allocations:
            if not isinstance(alloc, mybir.MemoryLocationSet):
                continue
            name = alloc.memorylocations[0].name
            if alloc.kind == "ExternalInput":
                if name != partition_name and name != (
                    nc.dbg_addr.name if nc.dbg_addr else None
                ):
                    in_names.append(name)
            elif alloc.kind == "ExternalOutput":
                out_names.append(name)
                shape = tuple(alloc.tensor_shape)
                dtype = mybir.dt.np(alloc.dtype)
                out_avals.append(jax.core.ShapedArray(shape, dtype))
                zero_shapes.append((shape, dtype))

        self.in_names, self.out_names = in_names, out_names
        self.out_avals, self.zero_shapes = out_avals, zero_shapes
        n_params, n_outs = len(in_names), len(out_names)

        full_in_names = list(in_names) + list(out_names)
        if nc.dbg_addr is not None:
            full_in_names.append(nc.dbg_addr.name)
        if partition_name is not None:
            full_in_names.append(partition_name)
        has_dbg = nc.dbg_addr is not None

        def _body(*args):
            operands = list(args)
            if has_dbg:
                operands.append(np.zeros((1, 2), np.uint32))
            if partition_name is not None:
                operands.append(partition_id_tensor())
            return tuple(
                _bass_exec_p.bind(
                    *operands,
                    out_avals=tuple(out_avals),
                    in_names=tuple(full_in_names),
                    out_names=tuple(out_names),
                    lowering_input_output_aliases=(),
                    sim_require_finite=True,
                    sim_require_nnan=True,
                    nc=nc,
                )
            )

        donate = tuple(range(n_params, n_params + n_outs))
        devices = jax.devices()[:n_cores]
        mesh = Mesh(np.asarray(devices), ("core",))
        self._fn = jax.jit(
            shard_map(
                _body,
                mesh=mesh,
                in_specs=(PartitionSpec("core"),) * (n_params + n_outs),
                out_specs=(PartitionSpec("core"),) * n_outs,
                check_rep=False,
            ),
            donate_argnums=donate,
            keep_unused=True,
        )

    def prep_inputs(self, in_maps):
        n = self.n_cores
        concat = [
            np.concatenate([np.asarray(in_maps[c][name]) for c in range(n)], axis=0)
            for name in self.in_names
        ]
        return [jax.device_put(a) for a in concat]

    def _zeros(self):
        return [
            np.zeros((self.n_cores * s[0], *s[1:]), d) for s, d in self.zero_shapes
        ]

    def run_prepped(self, dev_inputs):
        out = self._fn(*dev_inputs, *self._zeros())
        jax.block_until_ready(out)
        return out

    def run(self, in_maps):
        out_arrs = self.run_prepped(self.prep_inputs(in_maps))
        n = self.n_cores
        return [
            {
                name: np.asarray(out_arrs[i]).reshape(n, *self.out_avals[i].shape)[c]
                for i, name in enumerate(self.out_names)
            }
            for c in range(n)
        ]


_PERM = np.concatenate([np.arange(0, HD, 2), np.arange(1, HD, 2)])  # de-interleave


def make_in_maps(x, cos, sin, wqkv, wo):
    x = np.asarray(x, np.float32)
    cos = np.asarray(cos, np.float32)
    sin = np.asarray(sin, np.float32)
    wqkv = np.asarray(wqkv, np.float32)
    wo = np.asarray(wo, np.float32)

    xt = np.ascontiguousarray(
        x[0].T.reshape(DO, 128, S).transpose(1, 0, 2)
    )  # [128, DO, S]

    cosT, sinT = cos.T, sin.T  # [32, S]
    ropea = np.ascontiguousarray(np.tile(cosT, (2, 1)))  # [64, S]
    ropeb = np.ascontiguousarray(np.concatenate([-sinT, sinT], axis=0))

    pp, ff = np.arange(128)[:, None], np.arange(SW)[None, :]
    masks = np.stack(
        [(ff >= 128 * j + pp).astype(np.float32) for j in range(4)], axis=1
    )  # [128, 4, SW]

    ident = np.eye(HD, dtype=np.float32)
    onescol = np.ones((1, HD), np.float32)
    vones = np.ones((128, NSB, 1), np.float32)

    in_maps = []
    for i in range(N_CORES):
        wq = wqkv[i * EQ : (i + 1) * EQ].reshape(HQ, HD, D)[:, _PERM, :].reshape(
            EQ, D
        )
        wk = wqkv[NH * HD + i * HD : NH * HD + (i + 1) * HD][_PERM]
        wv = wqkv[NH * HD + NKV * HD + i * HD : NH * HD + NKV * HD + (i + 1) * HD]
        wcore = np.concatenate([wq, wk, wv], axis=0)  # [384, D]
        wt = np.ascontiguousarray(
            wcore.T.reshape(DO, 128, ECORE).transpose(1, 0, 2)
        )
        wos = wo[:, i * EQ : (i + 1) * EQ]  # [D, 256]
        wot = np.ascontiguousarray(
            wos.T.reshape(2, 128, D).transpose(1, 0, 2)
        )
        in_maps.append(
            {
                "xt": xt,
                "wt": wt,
                "wot": wot,
                "ropea": ropea,
                "ropeb": ropeb,
                "masks": masks,
                "ident": ident,
                "onescol": onescol,
                "vones": vones,
            }
        )
    return in_maps


_CACHE = {}


def get_compiled(loop: int = 1) -> _CompiledSpmd:
    if loop not in _CACHE:
        _CACHE[loop] = _CompiledSpmd(_build(loop))
    return _CACHE[loop]


def kernel(x, cos, sin, wqkv, wo):
    comp = get_compiled(1)
    in_maps = make_in_maps(x, cos, sin, wqkv, wo)
    results = comp.run(in_maps)
    acc = results[0]["out"].astype(np.float64)
    for c in range(1, N_CORES):
        acc += results[c]["out"]
    return acc.astype(np.float32).reshape(B, S, D)


# revision 5
# speedup vs baseline: 1.2512x; 1.2512x over previous
"""Trainium2 Bass kernel for nn_Attention_18176301596931.

Dense GQA attention layer (B=1, S=2048, D=2048, 32 Q heads / 8 KV heads,
HD=64, interleaved RoPE, causal softmax) tensor-parallel over 8 NeuronCores:
core i owns Q heads 4i..4i+3 and KV head i. Each core computes its heads'
QKV projection, RoPE, attention, and a partial output projection over its
256 columns of wo; the host sums the 8 partial outputs.

All matmuls run as float32r (full-rate on TRN2 when the moving free dim is
>= 256) with fp32 PSUM accumulation:
  - qkvT[e,s]   = wqkvT.T @ xT          (contraction over d, 16 chunks)
  - scoresT     = kT_chunk.T @ qT_win   ([sk=128, sq=512] blocks)
  - pT          = exp(0.125*scoresT)    (ACT, straight from PSUM; causal
                                         mask = 0/1 multiply, diag blocks)
  - yT_aug      = v_aug.T @ pT          (v augmented with a ones column so
                                         row 64 accumulates softmax sums)
  - out_part    = yT.T @ woT            (partial over this core's e range)

Softmax skips max-subtraction (logits ~ N(0,1), exp is safe in fp32; the
result is mathematically identical). RoPE pairs are de-interleaved by
permuting wqkv's q/k rows host-side, so on-device RoPE is three
128-partition DVE ops per tile (out = t*A + swap(t)*B).
"""

import numpy as np
import jax
from jax.sharding import Mesh, PartitionSpec
from jax.experimental.shard_map import shard_map

import concourse.bass as bass
import concourse.mybir as mybir
import concourse.tile as tile
from concourse.bacc import Bacc
from concourse import bass2jax
from concourse.bass2jax import (
    _bass_exec_p,
    install_neuronx_cc_hook,
    partition_id_tensor,
)

F32 = mybir.dt.float32
F32R = mybir.dt.float32r
F16 = mybir.dt.float16

B, S, D = 1, 2048, 2048
NH, NKV, HD = 32, 8, 64
N_CORES = 8
HQ = NH // N_CORES          # 4 q heads per core
EQ = HQ * HD                # 256 q columns per core
ECORE = EQ + 2 * HD         # 384 qkv columns per core
DO = D // 128               # 16 contraction chunks
SW = 512                    # seq window (matmul moving dim)
NG = S // SW                # 4 groups
NSB = S // 128              # 16 sk blocks
SCALE = 1.0 / np.sqrt(HD)


def _build(loop: int = 1) -> bass.Bass:
    nc = Bacc()
    x_ext = nc.declare_dram_parameter("xt", [128, DO, S], F16, isOutput=False)
    w_ext = nc.declare_dram_parameter("wt", [128, DO, ECORE], F16, isOutput=False)
    wo_ext = nc.declare_dram_parameter("wot", [128, 2, D], F16, isOutput=False)
    a_ext = nc.declare_dram_parameter("ropea", [HD, S], F32R, isOutput=False)
    b_ext = nc.declare_dram_parameter("ropeb", [HD, S], F32R, isOutput=False)
    m_ext = nc.declare_dram_parameter("masks", [128, 4, SW], F32R, isOutput=False)
    id_ext = nc.declare_dram_parameter("ident", [HD, HD], F32R, isOutput=False)
    oc_ext = nc.declare_dram_parameter("onescol", [1, HD], F32R, isOutput=False)
    vo_ext = nc.declare_dram_parameter("vones", [128, NSB, 1], F32R, isOutput=False)
    out_ext = nc.declare_dram_parameter("out", [S, D], F16, isOutput=True)

    W1 = 256                  # phase-1 seq window (keeps xs buffers small)
    NW1 = S // W1

    with (
        nc.allow_low_precision(reason="float32r storage is intentional"),
        tile.TileContext(nc) as tc,
        tc.tile_pool(name="const", bufs=1) as constp,
        tc.tile_pool(name="persist", bufs=1) as persist,
        tc.tile_pool(name="xs", bufs=2) as xsp,
        tc.tile_pool(name="work", bufs=2) as work,
        tc.tile_pool(name="pt", bufs=3) as ptp,
        tc.tile_pool(name="outp", bufs=2) as outp,
        tc.tile_pool(name="ps", bufs=4, space="PSUM") as psp,
        tc.tile_pool(name="psy", bufs=2, space="PSUM") as psyp,
        tc.tile_pool(name="psb", bufs=1, space="PSUM") as psbp,
        tc.tile_pool(name="psv", bufs=1, space="PSUM") as psvp,
    ):
        ident = constp.tile([HD, HD], F32R)
        onescol = constp.tile([1, HD], F32R)
        ropea = constp.tile([HD, S], F32R)
        ropeb = constp.tile([HD, S], F32R)
        masks = constp.tile([128, 4, SW], F32R)
        wot = constp.tile([128, 2, D], F16)
        wt_sb = constp.tile([128, DO, ECORE], F16)
        nc.sync.dma_start(ident[:], id_ext[:])
        nc.sync.dma_start(onescol[:], oc_ext[:])
        nc.sync.dma_start(ropea[:], a_ext[:])
        nc.sync.dma_start(ropeb[:], b_ext[:])
        nc.sync.dma_start(masks[:], m_ext[:])
        nc.sync.dma_start(wot[:], wo_ext[:])
        nc.sync.dma_start(wt_sb[:], w_ext[:])

        def body(_i=None):
            # ---- Phase 1: QKV projection with RoPE fused into the PSUM
            # eviction. PSUM-source ops may shift base partitions, so the
            # real/imag swap halves are read straight out of PSUM. ----
            qh = [persist.tile([HD, S], F32R, name=f"qh{h}") for h in range(HQ)]
            kT = persist.tile([HD, S], F32R, name="kT")
            vT = persist.tile([HD, S], F32R, name="vT")

            def rope_evict(dst_cols, ps, r, w1cols):
                # dst = ps[r:r+64]*A + swap(ps[r:r+64])*B
                sw_t = work.tile([HD, W1], F32R, name="swtmp")
                nc.vector.tensor_mul(
                    out=sw_t[0:32, :], in0=ps[r + 32 : r + 64, :],
                    in1=ropeb[0:32, w1cols],
                )
                nc.vector.tensor_mul(
                    out=sw_t[32:64, :], in0=ps[r : r + 32, :],
                    in1=ropeb[32:64, w1cols],
                )
                nc.vector.tensor_mul(
                    out=dst_cols, in0=ps[r : r + HD, :], in1=ropea[:, w1cols]
                )
                nc.vector.tensor_add(out=dst_cols, in0=dst_cols, in1=sw_t[:])

            for w in range(NW1):
                cols = slice(w * W1, (w + 1) * W1)
                xs = xsp.tile([128, DO, W1], F16, name="xs")
                nc.sync.dma_start(xs[:], x_ext[:, :, cols])
                for e in range(3):
                    ps = psp.tile([128, SW], F32, name="mm")[:, :W1]
                    for ko in range(DO):
                        nc.tensor.matmul(
                            ps,
                            wt_sb[:, ko, e * 128 : (e + 1) * 128],
                            xs[:, ko, :],
                            start=(ko == 0),
                            stop=(ko == DO - 1),
                        )
                    if e < 2:
                        for half in range(2):
                            h = 2 * e + half
                            rope_evict(qh[h][:, cols], ps, 64 * half, cols)
                    else:
                        rope_evict(kT[:, cols], ps, 0, cols)
                        nc.scalar.copy(vT[:, cols], ps[HD:128, :])

            # ---- Phase 2: transpose V into [sk, hd] layout + ones column ----
            v_sk = persist.tile([128, NSB, HD + 1], F32R, name="v_sk")
            nc.sync.dma_start(v_sk[:, :, HD : HD + 1], vo_ext[:])
            for c in range(NSB):
                pv = psvp.tile([128, HD], F32R, name="vt")
                nc.tensor.transpose(
                    pv[:], vT[:, c * 128 : (c + 1) * 128], ident[:]
                )
                nc.vector.tensor_copy(v_sk[:, c, 0:HD], pv[:])

            # ---- Phase 4: attention per (head, group) ----
            yT = [persist.tile([128, S], F16, name=f"yT{c}") for c in range(2)]
            for h in range(HQ):
                for g in range(NG):
                    qw = qh[h][:, g * SW : (g + 1) * SW]
                    nblk = 4 * (g + 1)
                    psy = psyp.tile([HD + 1, SW], F32, name="y")
                    for b in range(nblk):
                        pss = psp.tile([128, SW], F32, name="mm")
                        nc.tensor.matmul(
                            pss[:],
                            kT[:, b * 128 : (b + 1) * 128],
                            qw,
                            start=True,
                            stop=True,
                        )
                        pt = ptp.tile([128, SW], F32R, name="pt")
                        nc.scalar.activation(
                            pt[:],
                            pss[:],
                            mybir.ActivationFunctionType.Exp,
                            scale=float(SCALE),
                        )
                        j = b - (nblk - 4)
                        if j >= 0:
                            nc.gpsimd.tensor_mul(
                                out=pt[:], in0=pt[:], in1=masks[:, j, :]
                            )
                        nc.tensor.matmul(
                            psy[:],
                            v_sk[:, b, :],
                            pt[:],
                            start=(b == 0),
                            stop=(b == nblk - 1),
                        )
                    # normalize: yT = psy[0:64] * (1/sums) broadcast over hd
                    rec = work.tile([1, SW], F32R, name="rec")
                    nc.vector.reciprocal(rec[:], psy[HD : HD + 1, :])
                    psb = psbp.tile([HD, SW], F32, name="bc")
                    nc.tensor.matmul(psb[:], onescol[:], rec[:], start=True, stop=True)
                    bcs = work.tile([HD, SW], F32, name="bcs")
                    nc.scalar.copy(bcs[:], psb[:])
                    nc.vector.tensor_mul(
                        out=yT[h // 2][(h % 2) * HD : (h % 2) * HD + HD,
                                       g * SW : (g + 1) * SW],
                        in0=psy[0:HD, :],
                        in1=bcs[:],
                    )

            # ---- Phase 5: output projection (partial over this core's e) ----
            for sq in range(NSB):
                for do in range(NG):
                    pso = psp.tile([128, SW], F32, name="mm")
                    for c in range(2):
                        nc.tensor.matmul(
                            pso[:],
                            yT[c][:, sq * 128 : (sq + 1) * 128],
                            wot[:, c, do * SW : (do + 1) * SW],
                            start=(c == 0),
                            stop=(c == 1),
                        )
                    ot = outp.tile([128, SW], F16, name="ot")
                    if (sq + do) % 2 == 0:
                        nc.scalar.copy(ot[:], pso[:])
                    else:
                        nc.vector.tensor_copy(ot[:], pso[:])
                    nc.sync.dma_start(
                        out_ext[sq * 128 : (sq + 1) * 128, do * SW : (do + 1) * SW],
                        ot[:],
                    )

        if loop <= 1:
            body()
        else:
            with tc.For_i(0, loop, 1) as i:
                body(i)
    nc.finalize()
    return nc


class _CompiledSpmd:
    def __init__(self, nc: bass.Bass, n_cores: int = N_CORES):
        install_neuronx_cc_hook()
        self.nc = nc
        self.n_cores = n_cores
        partition_name = nc.partition_id_tensor.name if nc.partition_id_tensor else None

        in_names, out_names, out_avals, zero_shapes = [], [], [], []
        for alloc in nc.m.functions[0].allocations:
            if not isinstance(alloc, mybir.MemoryLocationSet):
                continue
            name = alloc.memorylocations[0].name
            if alloc.kind == "ExternalInput":
                if name != partition_name and name != (
                    nc.dbg_addr.name if nc.dbg_addr else None
                ):
                    in_names.append(name)
            elif alloc.kind == "ExternalOutput":
                out_names.append(name)
                shape = tuple(alloc.tensor_shape)
                dtype = mybir.dt.np(alloc.dtype)
                out_avals.append(jax.core.ShapedArray(shape, dtype))
                zero_shapes.append((shape, dtype))

        self.in_names, self.out_names = in_names, out_names
        self.out_avals, self.zero_shapes = out_avals, zero_shapes
        n_params, n_outs = len(in_names), len(out_names)

        full_in_names = list(in_names) + list(out_names)
        if nc.dbg_addr is not None:
            full_in_names.append(nc.dbg_addr.name)
        if partition_name is not None:
            full_in_names.append(partition_name)
        has_dbg = nc.dbg_addr is not None

        def _body(*args):
            operands = list(args)
            if has_dbg:
                operands.append(np.zeros((1, 2), np.uint32))
            if partition_name is not None:
                operands.append(partition_id_tensor())
            return tuple(
                _bass_exec_p.bind(
                    *operands,
                    out_avals=tuple(out_avals),
                    in_names=tuple(full_in_names),
                    out_names=tuple(out_names),
                    lowering_input_output_aliases=(),
                    sim_require_finite=True,
                    sim_require_nnan=True,
                    nc=nc,
                )
            )

        donate = tuple(range(n_params, n_params + n_outs))
        devices = jax.devices()[:n_cores]
        mesh = Mesh(np.asarray(devices), ("core",))
        self._fn = jax.jit(
            shard_map(
                _body,
                mesh=mesh,
                in_specs=(PartitionSpec("core"),) * (n_params + n_outs),
                out_specs=(PartitionSpec("core"),) * n_outs,
                check_rep=False,
            ),
            donate_argnums=donate,
            keep_unused=True,
        )

    def prep_inputs(self, in_maps):
        n = self.n_cores
        concat = [
            np.concatenate([np.asarray(in_maps[c][name]) for c in range(n)], axis=0)
            for name in self.in_names
        ]
        return [jax.device_put(a) for a in concat]

    def _zeros(self):
        return [
            np.zeros((self.n_cores * s[0], *s[1:]), d) for s, d in self.zero_shapes
        ]

    def run_prepped(self, dev_inputs):
        out = self._fn(*dev_inputs, *self._zeros())
        jax.block_until_ready(out)
        return out

    def run(self, in_maps):
        out_arrs = self.run_prepped(self.prep_inputs(in_maps))
        n = self.n_cores
        return [
            {
                name: np.asarray(out_arrs[i]).reshape(n, *self.out_avals[i].shape)[c]
                for i, name in enumerate(self.out_names)
            }
            for c in range(n)
        ]


_PERM = np.concatenate([np.arange(0, HD, 2), np.arange(1, HD, 2)])  # de-interleave


def make_in_maps(x, cos, sin, wqkv, wo):
    x = np.asarray(x, np.float32)
    cos = np.asarray(cos, np.float32)
    sin = np.asarray(sin, np.float32)
    wqkv = np.asarray(wqkv, np.float32)
    wo = np.asarray(wo, np.float32)

    xt = np.ascontiguousarray(
        x[0].T.reshape(DO, 128, S).transpose(1, 0, 2)
    ).astype(np.float16)  # [128, DO, S]

    cosT, sinT = cos.T, sin.T  # [32, S]
    ropea = np.ascontiguousarray(np.tile(cosT, (2, 1)))  # [64, S]
    ropeb = np.ascontiguousarray(np.concatenate([-sinT, sinT], axis=0))

    pp, ff = np.arange(128)[:, None], np.arange(SW)[None, :]
    masks = np.stack(
        [(ff >= 128 * j + pp).astype(np.float32) for j in range(4)], axis=1
    )  # [128, 4, SW]

    ident = np.eye(HD, dtype=np.float32)
    onescol = np.ones((1, HD), np.float32)
    vones = np.ones((128, NSB, 1), np.float32)

    in_maps = []
    for i in range(N_CORES):
        wq = wqkv[i * EQ : (i + 1) * EQ].reshape(HQ, HD, D)[:, _PERM, :].reshape(
            EQ, D
        )
        wk = wqkv[NH * HD + i * HD : NH * HD + (i + 1) * HD][_PERM]
        wv = wqkv[NH * HD + NKV * HD + i * HD : NH * HD + NKV * HD + (i + 1) * HD]
        wcore = np.concatenate([wq, wk, wv], axis=0)  # [384, D]
        wt = np.ascontiguousarray(
            wcore.T.reshape(DO, 128, ECORE).transpose(1, 0, 2)
        ).astype(np.float16)
        wos = wo[:, i * EQ : (i + 1) * EQ]  # [D, 256]
        wot = np.ascontiguousarray(
            wos.T.reshape(2, 128, D).transpose(1, 0, 2)
        ).astype(np.float16)
        in_maps.append(
            {
                "xt": xt,
                "wt": wt,
                "wot": wot,
                "ropea": ropea,
                "ropeb": ropeb,
                "masks": masks,
                "ident": ident,
                "onescol": onescol,
                "vones": vones,
            }
        )
    return in_maps


_CACHE = {}


def get_compiled(loop: int = 1) -> _CompiledSpmd:
    if loop not in _CACHE:
        _CACHE[loop] = _CompiledSpmd(_build(loop))
    return _CACHE[loop]


def kernel(x, cos, sin, wqkv, wo):
    comp = get_compiled(1)
    in_maps = make_in_maps(x, cos, sin, wqkv, wo)
    results = comp.run(in_maps)
    acc = results[0]["out"].astype(np.float32)
    for c in range(1, N_CORES):
        acc += results[c]["out"].astype(np.float32)
    return acc.astype(np.float32).reshape(B, S, D)


# revision 7
# speedup vs baseline: 1.3656x; 1.0914x over previous
"""Trainium2 Bass kernel for nn_Attention_18176301596931.

Dense GQA attention layer (B=1, S=2048, D=2048, 32 Q heads / 8 KV heads,
HD=64, interleaved RoPE, causal softmax) tensor-parallel over 8 NeuronCores:
core i owns Q heads 4i..4i+3 and KV head i. Each core computes its heads'
QKV projection, RoPE, attention, and a partial output projection over its
256 columns of wo; the host sums the 8 partial outputs.

All matmuls run as float32r (full-rate on TRN2 when the moving free dim is
>= 256) with fp32 PSUM accumulation:
  - qkvT[e,s]   = wqkvT.T @ xT          (contraction over d, 16 chunks)
  - scoresT     = kT_chunk.T @ qT_win   ([sk=128, sq=512] blocks)
  - pT          = exp(0.125*scoresT)    (ACT, straight from PSUM; causal
                                         mask = 0/1 multiply, diag blocks)
  - yT_aug      = v_aug.T @ pT          (v augmented with a ones column so
                                         row 64 accumulates softmax sums)
  - out_part    = yT.T @ woT            (partial over this core's e range)

Softmax skips max-subtraction (logits ~ N(0,1), exp is safe in fp32; the
result is mathematically identical). RoPE pairs are de-interleaved by
permuting wqkv's q/k rows host-side, so on-device RoPE is three
128-partition DVE ops per tile (out = t*A + swap(t)*B).
"""

import numpy as np
import jax
from jax.sharding import Mesh, PartitionSpec
from jax.experimental.shard_map import shard_map

import concourse.bass as bass
import concourse.mybir as mybir
import concourse.tile as tile
from concourse.bacc import Bacc
from concourse import bass2jax
from concourse.bass2jax import (
    _bass_exec_p,
    install_neuronx_cc_hook,
    partition_id_tensor,
)

F32 = mybir.dt.float32
F32R = mybir.dt.float32r
F16 = mybir.dt.float16

B, S, D = 1, 2048, 2048
NH, NKV, HD = 32, 8, 64
N_CORES = 8
HQ = NH // N_CORES          # 4 q heads per core
EQ = HQ * HD                # 256 q columns per core
ECORE = EQ + 2 * HD         # 384 qkv columns per core
DO = D // 128               # 16 contraction chunks
SW = 512                    # seq window (matmul moving dim)
NG = S // SW                # 4 groups
NSB = S // 128              # 16 sk blocks
SCALE = 1.0 / np.sqrt(HD)


def _build(loop: int = 1) -> bass.Bass:
    nc = Bacc()
    x_ext = nc.declare_dram_parameter("xt", [128, DO, S], F16, isOutput=False)
    w_ext = nc.declare_dram_parameter("wt", [128, DO, ECORE], F16, isOutput=False)
    wo_ext = nc.declare_dram_parameter("wot", [128, 2, D], F16, isOutput=False)
    a_ext = nc.declare_dram_parameter("ropea", [128, S], F32R, isOutput=False)
    b_ext = nc.declare_dram_parameter("ropeb", [128, S], F32R, isOutput=False)
    m_ext = nc.declare_dram_parameter("masks", [128, 4, SW], F32R, isOutput=False)
    id_ext = nc.declare_dram_parameter("ident", [HD, HD], F32R, isOutput=False)
    oc_ext = nc.declare_dram_parameter("onescol", [1, HD], F32R, isOutput=False)
    vo_ext = nc.declare_dram_parameter("vones", [128, NSB, 1], F32R, isOutput=False)
    out_ext = nc.declare_dram_parameter("out", [S, D], F16, isOutput=True)

    W1 = 256                  # phase-1 seq window
    WPG = SW // W1            # phase-1 windows per attention group

    with (
        nc.allow_low_precision(reason="float32r/fp16 storage is intentional"),
        tile.TileContext(nc) as tc,
        tc.tile_pool(name="const", bufs=1) as constp,
        tc.tile_pool(name="persist", bufs=1) as persist,
        tc.tile_pool(name="xs", bufs=2) as xsp,
        tc.tile_pool(name="work", bufs=2) as work,
        tc.tile_pool(name="pt", bufs=3) as ptp,
        tc.tile_pool(name="psqk", bufs=2, space="PSUM") as psqk,
        tc.tile_pool(name="pssc", bufs=2, space="PSUM") as pssc,
        tc.tile_pool(name="psy", bufs=2, space="PSUM") as psyp,
        tc.tile_pool(name="psb", bufs=1, space="PSUM") as psbp,
        tc.tile_pool(name="psv", bufs=1, space="PSUM") as psvp,
    ):
        ident = constp.tile([HD, HD], F32R)
        onescol = constp.tile([1, HD], F32R)
        ropea = constp.tile([128, S], F32R)
        ropeb = constp.tile([128, S], F32R)
        masks = constp.tile([128, 4, SW], F32R)
        wot = constp.tile([128, 2, D], F16)
        wt_sb = constp.tile([128, DO, ECORE], F16)
        nc.sync.dma_start(ident[:], id_ext[:])
        nc.sync.dma_start(onescol[:], oc_ext[:])
        nc.sync.dma_start(ropea[:], a_ext[:])
        nc.sync.dma_start(ropeb[:], b_ext[:])
        nc.sync.dma_start(masks[:], m_ext[:])
        nc.sync.dma_start(wot[:], wo_ext[:])
        nc.sync.dma_start(wt_sb[:], w_ext[:])

        def body(_i=None):
            # qq[t]: packed RoPE'd q head-pair tiles [128, S] (heads 2t, 2t+1,
            # each as [32 real, 32 imag]); kT2: RoPE'd k duplicated in both
            # partition halves so matmul base-alignment works for odd heads.
            qq = [persist.tile([128, S], F32R, name=f"qq{t}") for t in range(2)]
            kT2 = persist.tile([128, S], F32R, name="kT2")
            vT = persist.tile([HD, S], F32R, name="vT")
            v_sk = persist.tile([128, NSB, HD + 1], F32R, name="v_sk")
            nc.sync.dma_start(v_sk[:, :, HD : HD + 1], vo_ext[:])
            yT = [persist.tile([128, S], F16, name=f"yT{c}") for c in range(2)]

            def rope_evict(dst, ps, rows, cols):
                # dst[rows] = ps[rows]*A + swap32(ps[rows])*B, rows 0:64/0:128
                n32 = rows // 32
                sw_t = work.tile([128, W1], F32R, name="swtmp")
                for b32 in range(n32):
                    lo = b32 * 32
                    src = lo + 32 if b32 % 2 == 0 else lo - 32
                    nc.vector.tensor_copy(
                        sw_t[lo : lo + 32, :], ps[src : src + 32, :]
                    )
                nc.vector.tensor_mul(
                    out=sw_t[:rows, :], in0=sw_t[:rows, :], in1=ropeb[:rows, cols]
                )
                nc.vector.tensor_mul(
                    out=dst, in0=ps[:rows, :], in1=ropea[:rows, cols]
                )
                nc.gpsimd.tensor_add(out=dst, in0=dst, in1=sw_t[:rows, :])

            for g in range(NG):
                # ---- QKV projection for this group's seq windows ----
                for wi in range(WPG):
                    w = g * WPG + wi
                    cols = slice(w * W1, (w + 1) * W1)
                    xs = xsp.tile([128, DO, W1], F16, name="xs")
                    nc.sync.dma_start(xs[:], x_ext[:, :, cols])
                    for e in range(3):
                        ps = psqk.tile([128, SW], F32, name="qk")[:, :W1]
                        for ko in range(DO):
                            nc.tensor.matmul(
                                ps,
                                wt_sb[:, ko, e * 128 : (e + 1) * 128],
                                xs[:, ko, :],
                                start=(ko == 0),
                                stop=(ko == DO - 1),
                            )
                        if e < 2:
                            rope_evict(qq[e][:, cols], ps, 128, cols)
                        else:
                            rope_evict(kT2[0:HD, cols], ps, HD, cols)
                            nc.gpsimd.dma_start(
                                kT2[HD:128, cols], kT2[0:HD, cols]
                            )
                            nc.scalar.copy(vT[:, cols], ps[HD:128, :])
                # ---- V transpose for this group's sk chunks ----
                for c in range(4 * g, 4 * g + 4):
                    pv = psvp.tile([128, HD], F32R, name="vt")
                    nc.tensor.transpose(
                        pv[:], vT[:, c * 128 : (c + 1) * 128], ident[:]
                    )
                    nc.vector.tensor_copy(v_sk[:, c, 0:HD], pv[:])

                # ---- attention for all heads on group g ----
                nblk = 4 * (g + 1)
                for h in range(HQ):
                    base = (h % 2) * HD
                    qw = qq[h // 2][base : base + HD, g * SW : (g + 1) * SW]
                    psy = psyp.tile([HD + 1, SW], F32, name="y")
                    for b in range(nblk):
                        j = b - (nblk - 4)
                        sub = slice(128 * j, SW) if j > 0 else slice(0, SW)
                        pss = pssc.tile([128, SW], F32, name="sc")
                        nc.tensor.matmul(
                            pss[:],
                            kT2[base : base + HD, b * 128 : (b + 1) * 128],
                            qw,
                            start=True,
                            stop=True,
                        )
                        pt = ptp.tile([128, SW], F32R, name="pt")
                        nc.scalar.activation(
                            pt[:, sub],
                            pss[:, sub],
                            mybir.ActivationFunctionType.Exp,
                            scale=float(SCALE),
                        )
                        if j >= 0:
                            nc.gpsimd.tensor_mul(
                                out=pt[:, sub], in0=pt[:, sub],
                                in1=masks[:, j, sub],
                            )
                        nc.tensor.matmul(
                            psy[:, sub],
                            v_sk[:, b, :],
                            pt[:, sub],
                            start=(b == 0),
                            stop=(b == nblk - 1),
                        )
                    rec = work.tile([1, SW], F32R, name="rec")
                    nc.vector.reciprocal(rec[:], psy[HD : HD + 1, :])
                    psb = psbp.tile([HD, SW], F32, name="bc")
                    nc.tensor.matmul(psb[:], onescol[:], rec[:], start=True, stop=True)
                    bcs = work.tile([HD, SW], F32, name="bcs")
                    nc.scalar.copy(bcs[:], psb[:])
                    nc.vector.tensor_mul(
                        out=yT[h // 2][base : base + HD, g * SW : (g + 1) * SW],
                        in0=psy[0:HD, :],
                        in1=bcs[:],
                    )

            # ---- output projection (partial over this core's e range) ----
            for sq in range(NSB):
                for do in range(NG):
                    pso = psqk.tile([128, SW], F32, name="qk")
                    for c in range(2):
                        nc.tensor.matmul(
                            pso[:],
                            yT[c][:, sq * 128 : (sq + 1) * 128],
                            wot[:, c, do * SW : (do + 1) * SW],
                            start=(c == 0),
                            stop=(c == 1),
                        )
                    ot = work.tile([128, SW], F16, name="ot")
                    if (sq + do) % 2 == 0:
                        nc.scalar.copy(ot[:], pso[:])
                    else:
                        nc.vector.tensor_copy(ot[:], pso[:])
                    nc.sync.dma_start(
                        out_ext[sq * 128 : (sq + 1) * 128, do * SW : (do + 1) * SW],
                        ot[:],
                    )

        if loop <= 1:
            body()
        else:
            with tc.For_i(0, loop, 1) as i:
                body(i)
    nc.finalize()
    return nc


class _CompiledSpmd:
    def __init__(self, nc: bass.Bass, n_cores: int = N_CORES):
        install_neuronx_cc_hook()
        self.nc = nc
        self.n_cores = n_cores
        partition_name = nc.partition_id_tensor.name if nc.partition_id_tensor else None

        in_names, out_names, out_avals, zero_shapes = [], [], [], []
        for alloc in nc.m.functions[0].allocations:
            if not isinstance(alloc, mybir.MemoryLocationSet):
                continue
            name = alloc.memorylocations[0].name
            if alloc.kind == "ExternalInput":
                if name != partition_name and name != (
                    nc.dbg_addr.name if nc.dbg_addr else None
                ):
                    in_names.append(name)
            elif alloc.kind == "ExternalOutput":
                out_names.append(name)
                shape = tuple(alloc.tensor_shape)
                dtype = mybir.dt.np(alloc.dtype)
                out_avals.append(jax.core.ShapedArray(shape, dtype))
                zero_shapes.append((shape, dtype))

        self.in_names, self.out_names = in_names, out_names
        self.out_avals, self.zero_shapes = out_avals, zero_shapes
        n_params, n_outs = len(in_names), len(out_names)

        full_in_names = list(in_names) + list(out_names)
        if nc.dbg_addr is not None:
            full_in_names.append(nc.dbg_addr.name)
        if partition_name is not None:
            full_in_names.append(partition_name)
        has_dbg = nc.dbg_addr is not None

        def _body(*args):
            operands = list(args)
            if has_dbg:
                operands.append(np.zeros((1, 2), np.uint32))
            if partition_name is not None:
                operands.append(partition_id_tensor())
            return tuple(
                _bass_exec_p.bind(
                    *operands,
                    out_avals=tuple(out_avals),
                    in_names=tuple(full_in_names),
                    out_names=tuple(out_names),
                    lowering_input_output_aliases=(),
                    sim_require_finite=True,
                    sim_require_nnan=True,
                    nc=nc,
                )
            )

        donate = tuple(range(n_params, n_params + n_outs))
        devices = jax.devices()[:n_cores]
        mesh = Mesh(np.asarray(devices), ("core",))
        self._fn = jax.jit(
            shard_map(
                _body,
                mesh=mesh,
                in_specs=(PartitionSpec("core"),) * (n_params + n_outs),
                out_specs=(PartitionSpec("core"),) * n_outs,
                check_rep=False,
            ),
            donate_argnums=donate,
            keep_unused=True,
        )

    def prep_inputs(self, in_maps):
        n = self.n_cores
        concat = [
            np.concatenate([np.asarray(in_maps[c][name]) for c in range(n)], axis=0)
            for name in self.in_names
        ]
        return [jax.device_put(a) for a in concat]

    def _zeros(self):
        return [
            np.zeros((self.n_cores * s[0], *s[1:]), d) for s, d in self.zero_shapes
        ]

    def run_prepped(self, dev_inputs):
        out = self._fn(*dev_inputs, *self._zeros())
        jax.block_until_ready(out)
        return out

    def run(self, in_maps):
        out_arrs = self.run_prepped(self.prep_inputs(in_maps))
        n = self.n_cores
        return [
            {
                name: np.asarray(out_arrs[i]).reshape(n, *self.out_avals[i].shape)[c]
                for i, name in enumerate(self.out_names)
            }
            for c in range(n)
        ]


_PERM = np.concatenate([np.arange(0, HD, 2), np.arange(1, HD, 2)])  # de-interleave


def make_in_maps(x, cos, sin, wqkv, wo):
    x = np.asarray(x, np.float32)
    cos = np.asarray(cos, np.float32)
    sin = np.asarray(sin, np.float32)
    wqkv = np.asarray(wqkv, np.float32)
    wo = np.asarray(wo, np.float32)

    xt = np.ascontiguousarray(
        x[0].T.reshape(DO, 128, S).transpose(1, 0, 2)
    ).astype(np.float16)  # [128, DO, S]

    cosT, sinT = cos.T, sin.T  # [32, S]
    ropea = np.ascontiguousarray(np.tile(cosT, (4, 1)))  # [128, S]
    ropeb = np.ascontiguousarray(
        np.concatenate([-sinT, sinT, -sinT, sinT], axis=0)
    )

    pp, ff = np.arange(128)[:, None], np.arange(SW)[None, :]
    masks = np.stack(
        [(ff >= 128 * j + pp).astype(np.float32) for j in range(4)], axis=1
    )  # [128, 4, SW]

    ident = np.eye(HD, dtype=np.float32)
    onescol = np.ones((1, HD), np.float32)
    vones = np.ones((128, NSB, 1), np.float32)

    in_maps = []
    for i in range(N_CORES):
        wq = wqkv[i * EQ : (i + 1) * EQ].reshape(HQ, HD, D)[:, _PERM, :].reshape(
            EQ, D
        )
        wk = wqkv[NH * HD + i * HD : NH * HD + (i + 1) * HD][_PERM]
        wv = wqkv[NH * HD + NKV * HD + i * HD : NH * HD + NKV * HD + (i + 1) * HD]
        wcore = np.concatenate([wq, wk, wv], axis=0)  # [384, D]
        wt = np.ascontiguousarray(
            wcore.T.reshape(DO, 128, ECORE).transpose(1, 0, 2)
        ).astype(np.float16)
        wos = wo[:, i * EQ : (i + 1) * EQ]  # [D, 256]
        wot = np.ascontiguousarray(
            wos.T.reshape(2, 128, D).transpose(1, 0, 2)
        ).astype(np.float16)
        in_maps.append(
            {
                "xt": xt,
                "wt": wt,
                "wot": wot,
                "ropea": ropea,
                "ropeb": ropeb,
                "masks": masks,
                "ident": ident,
                "onescol": onescol,
                "vones": vones,
            }
        )
    return in_maps


_CACHE = {}


def get_compiled(loop: int = 1) -> _CompiledSpmd:
    if loop not in _CACHE:
        _CACHE[loop] = _CompiledSpmd(_build(loop))
    return _CACHE[loop]


def kernel(x, cos, sin, wqkv, wo):
    comp = get_compiled(1)
    in_maps = make_in_maps(x, cos, sin, wqkv, wo)
    results = comp.run(in_maps)
    acc = results[0]["out"].astype(np.float32)
    for c in range(1, N_CORES):
        acc += results[c]["out"].astype(np.float32)
    return acc.astype(np.float32).reshape(B, S, D)
